# revision 1
# baseline (speedup 1.0000x reference)
"""AttentionTSSA Trainium2 kernel.

Sharding: data-parallel over batch. B=8 -> one batch element per NeuronCore,
zero collectives. Host slices inputs / stacks outputs.

Per-core math (x: [N=4096, D=1024], heads h=16, head dim d=64):
  w[n, c]   = x @ W_qkv.T                   (c = hd flattened head*64+dd)
  s[c]      = sum_n w^2                     (col norms squared)
  logits[h,n] = temp[h] * sum_dd w^2[hd,n] / max(s[hd], 1e-24)
  Pi        = softmax_h(logits)
  dots[c]   = (sum_n Pi[h,n] * w^2[c,n]) / (sum_n Pi[h,n] + 1e-8)
  attn[c]   = 1 / (1 + dots)
  y         = (-(w * Pi_bcast) * attn_bcast) @ W_out.T + b_out

On-chip layout: w stored column-major [c(part), n(free)] as 8 tiles
[128, 4096] bf16, so every sum_n is a free-axis reduce and both big
matmuls (f32r, full PE rate) need no big transposes beyond x itself
(PE-transposed per chunk).
"""

import sys

sys.path.insert(0, "/opt/trn_rl_repo")

import numpy as np
import concourse.bacc as bacc
import concourse.tile as tile
from concourse import mybir
from concourse.bass_utils import run_bass_kernel_spmd
from concourse.masks import make_identity

F32 = mybir.dt.float32
F32R = mybir.dt.float32r
BF16 = mybir.dt.bfloat16
MUL = mybir.AluOpType.mult
ADD = mybir.AluOpType.add

B, N, D = 8, 4096, 1024
H, HD = 16, 64
P = 128
NT = D // P          # 8 col-partition tiles
CH = 512             # n-chunk
NCH = N // CH        # 8 chunks
MS = CH // P         # 4 n-subtiles per chunk


def build(reps=1, phases="ALSBM"):
    nc = bacc.Bacc()
    x_t = nc.dram_tensor("x", [N, D], F32, kind="ExternalInput")
    wq_t = nc.dram_tensor("wqT", [D, D], F32R, kind="ExternalInput")    # W_qkv.T
    wo_t = nc.dram_tensor("woT", [D, D], F32R, kind="ExternalInput")    # W_out.T
    temp_t = nc.dram_tensor("temp", [H, 1], F32, kind="ExternalInput")
    sel_t = nc.dram_tensor("sel", [NT, H, P], F32, kind="ExternalInput")
    selr_t = nc.dram_tensor("selr", [NT, H, P], F32R, kind="ExternalInput")
    bias_t = nc.dram_tensor("bout", [1, D], F32R, kind="ExternalInput")
    y_t = nc.dram_tensor("y", [N, D], F32, kind="ExternalOutput")

    with tile.TileContext(nc) as tc:
      for _rep in range(reps):
        with (
            tc.tile_pool(name="consts", bufs=1) as consts,
            tc.tile_pool(name="wmat", bufs=1) as wmat,
            tc.tile_pool(name="wsb", bufs=1) as wsb,
            tc.tile_pool(name="small", bufs=1) as small,
        ):
            # ---------- constants ----------
            ident = consts.tile([P, P], F32)
            make_identity(nc, ident)
            temp_sb = consts.tile([H, 1], F32)
            nc.sync.dma_start(out=temp_sb, in_=temp_t[:, :])
            bias_r = consts.tile([1, D], F32R)
            nc.sync.dma_start(out=bias_r, in_=bias_t[:, :])
            ones16 = consts.tile([H, 1], F32)
            nc.vector.memset(ones16, 1.0)
            ones1x16 = consts.tile([1, H], F32)
            nc.vector.memset(ones1x16, 1.0)
            ones1x128 = consts.tile([1, P], F32)
            nc.vector.memset(ones1x128, 1.0)
            ones1x128_r = consts.tile([1, P], F32R)
            nc.vector.tensor_copy(ones1x128_r, ones1x128)
            ones1x16r = consts.tile([1, H], F32R)
            nc.vector.tensor_copy(ones1x16r, ones1x16)

            # per-tile selectors (host constant): Sel01[t][j, p] = 1 iff j == 2t + p//64
            sel_f32 = []
            sel_r = []
            for t in range(NT):
                sf = consts.tile([H, P], F32, tag=f"self{t}", name=f"self{t}")
                nc.sync.dma_start(out=sf, in_=sel_t[t, :, :])
                sr = consts.tile([H, P], F32R, tag=f"selr{t}", name=f"selr{t}")
                nc.sync.dma_start(out=sr, in_=selr_t[t, :, :])
                sel_f32.append(sf)
                sel_r.append(sr)

            # weights (host pre-transposed); wq and wo share one slot (bufs=1):
            # wo is DMA'd after phase A frees wq
            wq_sb = wmat.tile([P, NT, D], F32R, tag="wm")
            for k in range(NT):
                nc.sync.dma_start(out=wq_sb[:, k, :], in_=wq_t[k * P : (k + 1) * P, :])

            # persistent big tensors
            w_tiles = [wsb.tile([P, N], BF16, tag=f"w{t}", name=f"w{t}") for t in range(NT)]
            s_strip = [small.tile([P, NCH], F32, tag=f"ss{t}", name=f"ss{t}") for t in range(NT)]
            d_strip = [small.tile([P, NCH], F32, tag=f"ds{t}", name=f"ds{t}") for t in range(NT)]

            # ---------- phase A: w = x @ WqkvT, s = sum_n w^2 ----------
            with (
                tc.tile_pool(name="achunk", bufs=2) as achunk,
                tc.tile_pool(name="scrA", bufs=3) as scrA,
                tc.tile_pool(name="psA", bufs=2, space="PSUM") as psA,
                tc.tile_pool(name="psTP", bufs=2, space="PSUM") as psTP,
            ):
                for c in range(NCH):
                    cs = slice(c * CH, (c + 1) * CH)
                    x_raw = achunk.tile([P, MS, D], F32, tag="xraw")
                    nc.sync.dma_start(
                        out=x_raw,
                        in_=x_t[cs, :].rearrange("(m p) i -> p m i", p=P),
                    )
                    xT = achunk.tile([P, NT, CH], F32R, tag="xT", bufs=1)
                    for k in range(NT):
                        tp_ps = psTP.tile([P, CH], F32, tag="tp")
                        for m in range(MS):
                            nc.tensor.transpose(
                                tp_ps[:, m * P : (m + 1) * P],
                                x_raw[:, m, k * P : (k + 1) * P],
                                ident,
                            )
                        nc.scalar.copy(out=xT[:, k, :], in_=tp_ps)
                    for t in range(NT):
                        w_ps = psA.tile([P, CH], F32, tag="mm1")
                        for k in range(NT):
                            nc.tensor.matmul(
                                w_ps,
                                wq_sb[:, k, t * P : (t + 1) * P],
                                xT[:, k, :],
                                start=(k == 0),
                                stop=(k == NT - 1),
                            )
                        nc.scalar.copy(out=w_tiles[t][:, cs], in_=w_ps)
                        junk = scrA.tile([P, CH], BF16, tag="junkA")
                        nc.vector.scalar_tensor_tensor(
                            out=junk,
                            in0=w_tiles[t][:, cs],
                            scalar=1.0,
                            in1=w_tiles[t][:, cs],
                            op0=MUL,
                            op1=MUL,
                            accum_out=s_strip[t][:, c : c + 1],
                        )

            # W_out.T load (reuses wq's slot; overlaps softmax phases)
            wo_sb = wmat.tile([P, NT, D], F32R, tag="wm")
            for k in range(NT):
                nc.sync.dma_start(out=wo_sb[:, k, :], in_=wo_t[k * P : (k + 1) * P, :])

            # softmax-side pool opens only after phase A frees its space
            soft = tc.alloc_tile_pool(name="soft", bufs=1)

            # ---------- stats 1: inv_temp, L_big ----------
            lbig = []
            with tc.tile_pool(name="psS1", bufs=2, space="PSUM") as psS1:
                for t in range(NT):
                    s_all = small.tile([P, 1], F32, tag=f"sall{t}")
                    nc.vector.reduce_sum(s_all, s_strip[t], axis=mybir.AxisListType.X)
                    nc.vector.tensor_scalar_max(out=s_all, in0=s_all, scalar1=1e-24)
                    rcp = small.tile([P, 1], F32, tag=f"rcp{t}")
                    nc.vector.reciprocal(rcp, s_all)
                    tb_ps = psS1.tile([P, 1], F32, tag="tb")
                    nc.tensor.matmul(tb_ps, sel_f32[t], temp_sb, start=True, stop=True)
                    inv_t = small.tile([P, 1], F32, tag=f"invt{t}")
                    nc.vector.tensor_mul(inv_t, rcp, tb_ps)
                    lb = small.tile([P, H], BF16, tag=f"lbig{t}")
                    nc.vector.memset(lb, 0.0)
                    nc.vector.tensor_copy(lb[0:HD, 2 * t : 2 * t + 1], inv_t[0:HD, :])
                    nc.vector.tensor_copy(
                        lb[HD:P, 2 * t + 1 : 2 * t + 2], inv_t[HD:P, :]
                    )
                    lbig.append(lb)

            # ---------- phase L: logits[h, n] (squares on DVE, copies on ACT) ----------
            if "L" not in phases:
                for t in range(NT):
                    nc.gpsimd.dma_start(
                        out=y_t[t * CH : (t + 1) * CH, :].rearrange(
                            "(m p) i -> p m i", p=P
                        ),
                        in_=w_tiles[t].rearrange("p (m i) -> p m i", i=D),
                    )
                nc.sync.dma_start(out=y_t[0:P, 0:NCH], in_=s_strip[0])
                soft.release()
                continue
            logits = soft.tile([H, N], F32R, tag="logits")
            with (
                tc.tile_pool(name="scrL", bufs=3) as scrL,
                tc.tile_pool(name="psL", bufs=2, space="PSUM") as psL,
            ):
                for c in range(NCH):
                    cs = slice(c * CH, (c + 1) * CH)
                    lg_ps = psL.tile([H, CH], F32, tag="lg")
                    for t in range(NT):
                        w2t = scrL.tile([P, CH], BF16, tag="w2t")
                        nc.vector.tensor_mul(
                            w2t, w_tiles[t][:, cs], w_tiles[t][:, cs]
                        )
                        nc.tensor.matmul(
                            lg_ps, lbig[t], w2t, start=(t == 0), stop=(t == NT - 1)
                        )
                    nc.scalar.copy(out=logits[:, cs], in_=lg_ps)

            # ---------- softmax over h, log-sum-exp form ----------
            if "S" not in phases:
                nc.sync.dma_start(
                    out=y_t[0 : H * MS, :].rearrange("(m p) i -> p m i", p=H),
                    in_=logits.bitcast(F32).rearrange("p (m i) -> p m i", i=D),
                )
                soft.release()
                continue
            # Pi = exp(logits - ln(sum_h exp(logits))); avoids a 4096-wide
            # iterative reciprocal on DVE.
            epool = tc.alloc_tile_pool(name="epool", bufs=1)
            e_hn = epool.tile([H, N], F32R, tag="ehn")
            nc.scalar.activation(
                out=e_hn, in_=logits.bitcast(F32),
                func=mybir.ActivationFunctionType.Exp,
            )
            ones16r = consts.tile([H, 1], F32R)
            nc.vector.tensor_copy(ones16r, ones16)
            # Pi overwrites logits in place (exp(logits - lnS) reads+writes same tile)
            pi_hn = logits
            sume_row = small.tile([1, N], F32, tag="sumerow")
            lns_row = small.tile([1, N], F32R, tag="lnsrow")
            # Few wide ops instead of many small per-chunk ops: each
            # cross-engine hop costs ~1us of sync latency on HW.
            with tc.tile_pool(name="psSM", bufs=1, space="PSUM") as psSM:
                se_ps = psSM.tile([1, N], F32, tag="sm_big")
                for c in range(NCH):
                    cs = slice(c * CH, (c + 1) * CH)
                    nc.tensor.matmul(
                        se_ps[:, cs], ones16r, e_hn[:, cs], start=True, stop=True
                    )
                nc.scalar.copy(out=sume_row, in_=se_ps)
                nc.scalar.activation(
                    out=lns_row, in_=sume_row, func=mybir.ActivationFunctionType.Ln
                )
                lnb_ps = psSM.tile([H, N], F32, tag="sm_big")
                for c in range(NCH):
                    cs = slice(c * CH, (c + 1) * CH)
                    nc.tensor.matmul(
                        lnb_ps[:, cs], ones1x16r, lns_row[:, cs], start=True, stop=True
                    )
                nc.vector.tensor_sub(logits, logits.bitcast(F32), lnb_ps)
                nc.scalar.activation(
                    out=pi_hn,
                    in_=logits.bitcast(F32),
                    func=mybir.ActivationFunctionType.Exp,
                )

            epool.release()

            sumpi = small.tile([H, 1], F32, tag="sumpi")
            nc.vector.reduce_sum(
                sumpi, pi_hn.bitcast(F32), axis=mybir.AxisListType.X
            )
            nc.vector.tensor_scalar_add(out=sumpi, in0=sumpi, scalar1=1e-8)
            ispi = small.tile([H, 1], F32, tag="ispi")
            nc.vector.reciprocal(ispi, sumpi)

            # ---------- phase B (merged): u = w*Pi_b overwrites w; dots ----------
            if "B" not in phases:
                nc.sync.dma_start(
                    out=y_t[0 : H * MS, :].rearrange("(m p) i -> p m i", p=H),
                    in_=pi_hn.bitcast(F32).rearrange("p (m i) -> p m i", i=D),
                )
                nc.sync.dma_start(out=y_t[H * MS : H * MS + H, 0:1], in_=ispi)
                soft.release()
                continue
            # 1024-wide chunks to amortize fixed per-op costs
            CHB = 1024
            NCHB = N // CHB
            with (
                tc.tile_pool(name="scrB", bufs=4) as scrB,
                tc.tile_pool(name="psB1", bufs=3, space="PSUM") as psB1,
            ):
                for c in range(NCHB):
                    cs = slice(c * CHB, (c + 1) * CHB)
                    for t in range(NT):
                        pib_ps = psB1.tile([P, CHB], F32, tag="pib")
                        for hh in range(CHB // CH):
                            nc.tensor.matmul(
                                pib_ps[:, hh * CH : (hh + 1) * CH],
                                sel_r[t],
                                pi_hn[:, c * CHB + hh * CH : c * CHB + (hh + 1) * CH],
                                start=True,
                                stop=True,
                            )
                        pib_sb = scrB.tile([P, CHB], BF16, tag="pibsb")
                        nc.scalar.copy(out=pib_sb, in_=pib_ps)
                        u_tmp = scrB.tile([P, CHB], BF16, tag="utmp")
                        nc.vector.tensor_mul(u_tmp, w_tiles[t][:, cs], pib_sb)
                        junk = scrB.tile([P, CHB], BF16, tag="junkB")
                        nc.vector.scalar_tensor_tensor(
                            out=junk,
                            in0=u_tmp,
                            scalar=1.0,
                            in1=w_tiles[t][:, cs],
                            op0=MUL,
                            op1=MUL,
                            accum_out=d_strip[t][:, c : c + 1],
                        )
                        # u overwrites w in place (WAR on the STT above)
                        nc.gpsimd.tensor_copy(out=w_tiles[t][:, cs], in_=u_tmp)

            # ---------- stats 2: attn; W' = -attn * WoutT (bf16) ----------
            wob, _wob_free = tc.tile([P, NT, D], BF16, name="wob")
            bias_bf = consts.tile([1, D], BF16)
            nc.vector.tensor_copy(bias_bf, bias_r.bitcast(F32))
            ones1x128b = consts.tile([1, P], BF16)
            nc.vector.memset(ones1x128b, 1.0)
            with tc.tile_pool(name="psS2", bufs=2, space="PSUM") as psS2:
                for t in range(NT):
                    isp_ps = psS2.tile([P, 1], F32, tag="isp")
                    nc.tensor.matmul(isp_ps, sel_f32[t], ispi, start=True, stop=True)
                    dots = small.tile([P, 1], F32, tag=f"dots{t}")
                    nc.vector.reduce_sum(
                        dots, d_strip[t][:, 0 : N // 1024], axis=mybir.AxisListType.X
                    )
                    nc.vector.tensor_mul(dots, dots, isp_ps)
                    nc.vector.tensor_scalar_add(out=dots, in0=dots, scalar1=1.0)
                    attn = small.tile([P, 1], F32, tag=f"attn{t}")
                    nc.vector.reciprocal(attn, dots)
                    nc.vector.tensor_scalar_mul(out=attn, in0=attn, scalar1=-1.0)
                    nc.vector.tensor_scalar_mul(
                        out=wob[:, t, :],
                        in0=wo_sb[:, t, :].bitcast(F32),
                        scalar1=attn,
                    )

            # ---------- phase MM2: y = u.T @ W' + b (dense PE) ----------
            if "M" not in phases:
                for t in range(NT):
                    nc.gpsimd.dma_start(
                        out=y_t[t * CH : (t + 1) * CH, :].rearrange(
                            "(m p) i -> p m i", p=P
                        ),
                        in_=w_tiles[t].rearrange("p (m i) -> p m i", i=D),
                    )
                nc.gpsimd.dma_start(
                    out=y_t[0:D, :].rearrange("(t p) i -> p t i", p=P),
                    in_=wob,
                )
                _wob_free()
                soft.release()
                continue
            # bias materialized [128, D] once: bias broadcast over partitions
            bias_sb, _bias_free = tc.tile([P, D], F32, name="bias_sb")
            with tc.tile_pool(name="psBb", bufs=1, space="PSUM") as psBb:
                bb_ps = psBb.tile([P, D], F32, tag="bb")
                for oh in range(2):
                    os_ = slice(oh * CH, (oh + 1) * CH)
                    nc.tensor.matmul(
                        bb_ps[:, os_],
                        ones1x128b,
                        bias_bf[:, os_],
                        start=True,
                        stop=True,
                    )
                nc.scalar.copy(out=bias_sb, in_=bb_ps)

            # MM2: pure PE accumulation; PSUM evicted by DVE with fused
            # bias add; output double-buffered at half-chunk granularity
            with (
                tc.tile_pool(name="och", bufs=2) as och,
                tc.tile_pool(name="psMM2", bufs=4, space="PSUM") as psMM2,
            ):
                for c in range(NCH):
                    for half in range(2):
                        outf = och.tile([P, MS // 2, D], F32, tag="outf")
                        for mh in range(MS // 2):
                            m = half * (MS // 2) + mh
                            ms_ = slice(c * CH + m * P, c * CH + (m + 1) * P)
                            for oh in range(2):
                                os_ = slice(oh * CH, (oh + 1) * CH)
                                f_ps = psMM2.tile([P, CH], F32, tag="mm2")
                                for t in range(NT):
                                    nc.tensor.matmul(
                                        f_ps,
                                        w_tiles[t][:, ms_],
                                        wob[:, t, os_],
                                        start=(t == 0),
                                        stop=(t == NT - 1),
                                    )
                                nc.vector.scalar_tensor_tensor(
                                    out=outf[:, mh, os_],
                                    in0=f_ps,
                                    scalar=1.0,
                                    in1=bias_sb[:, os_],
                                    op0=MUL,
                                    op1=ADD,
                                )
                        nc.sync.dma_start(
                            out=y_t[
                                c * CH + half * CH // 2 : c * CH + (half + 1) * CH // 2,
                                :,
                            ].rearrange("(m p) i -> p m i", p=P),
                            in_=outf,
                        )
            _bias_free()
            _wob_free()
            soft.release()

    if not nc.is_finalized():
        nc.finalize()
    return nc


_NC_CACHE = None
_LAST_IN_MAPS = None
_RUNNER = None


def _make_runner(nc, n_cores):
    """Like bass2jax.run_bass_via_pjrt but with the jitted callable cached,
    so repeat calls don't re-trace/re-compile the XLA wrapper."""
    import jax
    from jax.experimental.shard_map import shard_map
    from jax.sharding import Mesh, PartitionSpec
    from concourse import mybir as _mybir
    from concourse.bass2jax import (
        _bass_exec_p,
        install_neuronx_cc_hook,
        partition_id_tensor,
    )

    install_neuronx_cc_hook()

    partition_name = nc.partition_id_tensor.name if nc.partition_id_tensor else None
    in_names, out_names, out_avals, zero_outs = [], [], [], []
    for alloc in nc.m.functions[0].allocations:
        if not isinstance(alloc, _mybir.MemoryLocationSet):
            continue
        name = alloc.memorylocations[0].name
        if alloc.kind == "ExternalInput":
            if name != partition_name:
                in_names.append(name)
        elif alloc.kind == "ExternalOutput":
            shape = tuple(alloc.tensor_shape)
            dtype = _mybir.dt.np(alloc.dtype)
            out_names.append(name)
            out_avals.append(jax.core.ShapedArray(shape, dtype))
            zero_outs.append(np.zeros(shape, dtype))
    n_params = len(in_names)
    n_outs = len(out_names)
    all_in_names = in_names + out_names + (
        [partition_name] if partition_name else []
    )
    donate = tuple(range(n_params, n_params + n_outs))

    def _body(*args):
        operands = list(args)
        if partition_name is not None:
            operands.append(partition_id_tensor())
        outs = _bass_exec_p.bind(
            *operands,
            out_avals=tuple(out_avals),
            in_names=tuple(all_in_names),
            out_names=tuple(out_names),
            lowering_input_output_aliases=(),
            sim_require_finite=True,
            sim_require_nnan=True,
            nc=nc,
        )
        return tuple(outs)

    devices = jax.devices()[:n_cores]
    mesh = Mesh(np.asarray(devices), ("core",))
    in_specs = (PartitionSpec("core"),) * (n_params + n_outs)
    out_specs = (PartitionSpec("core"),) * n_outs
    sharded = jax.jit(
        shard_map(
            _body, mesh=mesh, in_specs=in_specs, out_specs=out_specs, check_rep=False
        ),
        donate_argnums=donate,
        keep_unused=True,
    )

    def run(in_maps):
        concat_in = [
            np.concatenate([np.asarray(m[name]) for m in in_maps], axis=0)
            for name in in_names
        ]
        concat_zeros = [
            np.zeros((n_cores * z.shape[0], *z.shape[1:]), z.dtype)
            for z in zero_outs
        ]
        out_arrs = sharded(*concat_in, *concat_zeros)
        return {
            name: np.asarray(out_arrs[i]).reshape(n_cores, *out_avals[i].shape)
            for i, name in enumerate(out_names)
        }

    run.sharded = sharded
    run.meta = (in_names, out_names, out_avals, n_params, n_outs)
    return run


def kernel(x, W_qkv, temp, W_out, b_out):
    global _NC_CACHE, _RUNNER
    if _NC_CACHE is None:
        _NC_CACHE = build()
        _RUNNER = _make_runner(_NC_CACHE, B)
    nc = _NC_CACHE

    x = np.asarray(x, dtype=np.float32)
    wqT = np.ascontiguousarray(np.asarray(W_qkv, dtype=np.float32).T)
    woT = np.ascontiguousarray(np.asarray(W_out, dtype=np.float32).T)
    temp = np.ascontiguousarray(np.asarray(temp, dtype=np.float32).reshape(H, 1))
    bout = np.ascontiguousarray(np.asarray(b_out, dtype=np.float32).reshape(1, D))

    sel = np.zeros((NT, H, P), dtype=np.float32)
    for t in range(NT):
        sel[t, 2 * t, 0:HD] = 1.0
        sel[t, 2 * t + 1, HD:P] = 1.0

    core_ids = list(range(B))
    in_maps = [
        {"x": np.ascontiguousarray(x[i]), "wqT": wqT, "woT": woT,
         "temp": temp, "bout": bout, "sel": sel, "selr": sel}
        for i in core_ids
    ]
    global _LAST_IN_MAPS
    _LAST_IN_MAPS = in_maps
    out = _RUNNER(in_maps)
    return out["y"]


if __name__ == "__main__":
    rng = np.random.default_rng(0)
    x = rng.standard_normal((B, N, D), dtype=np.float32)
    W_qkv = (rng.standard_normal((D, D), dtype=np.float32) * 0.02).astype(np.float32)
    temp = np.ones((H, 1), dtype=np.float32)
    W_out = (rng.standard_normal((D, D), dtype=np.float32) * 0.02).astype(np.float32)
    b_out = np.zeros((D,), dtype=np.float32)
    y = kernel(x=x, W_qkv=W_qkv, temp=temp, W_out=W_out, b_out=b_out)
    print("kernel ran, y shape", y.shape, "mean abs", np.abs(y).mean())



# revision 19
# speedup vs baseline: 1.3717x; 1.3717x over previous
"""AttentionTSSA Trainium2 kernel (v2).

Sharding: data-parallel over batch. B=8 -> one batch element per NeuronCore,
zero collectives. Host slices inputs / stacks outputs, and pre-transposes
x to x.T in bf16 so the kernel needs no PE transposes at all.

Per-core math (x: [N=4096, D=1024], heads h=16, head dim d=64):
  w[c, n]   = (x @ W_qkv.T).T                (c = hd flattened head*64+dd)
  s[c]      = sum_n w^2
  logits[h,n] = temp[h] * sum_dd w^2[hd,n] / max(s[hd], 1e-24)
  Pi        = softmax_h(logits)
  dots[c]   = (sum_n Pi[h,n] * w^2[c,n]) / (sum_n Pi[h,n] + 1e-8)
  u[c,n]    = -w * Pi_bcast / (1 + dots)
  y         = u.T @ W_out.T + b_out

Layout tricks:
  - w, w^2 stored column-major [c(part), n(free)] as 8 tiles [128, 4096] bf16.
  - logits/Pi live in a banded [128, 512] layout: partition p = 16*j + h
    (j = n-chunk of 512, h = head) so every softmax op is a full-width
    single instruction instead of [16, 4096] strips.
  - Banded selector matrices (host constants) implement the h<->c scatter/
    gather as K=128 matmuls with no partition-offset tricks.
  - attn is folded into u via one scalar_tensor_tensor per tile chunk; MM2
    streams u chunks immediately after they are produced (PE never idles
    long enough for a HAM re-throttle).
"""

import sys

sys.path.insert(0, "/opt/trn_rl_repo")

import numpy as np
import concourse.bacc as bacc
import concourse.tile as tile
from concourse import mybir
from concourse.bass_utils import run_bass_kernel_spmd

F32 = mybir.dt.float32
F32R = mybir.dt.float32r
BF16 = mybir.dt.bfloat16
MUL = mybir.AluOpType.mult
EXP = mybir.ActivationFunctionType.Exp
LN = mybir.ActivationFunctionType.Ln

B, N, D = 8, 4096, 1024
H, HD = 16, 64
P = 128
NT = D // P          # 8 c-tiles
CH = 512             # n-chunk
NCH = N // CH        # 8 chunks


def build():
    nc = bacc.Bacc()
    xT_t = nc.dram_tensor("xT", [D, N], BF16, kind="ExternalInput")      # x.T
    wq_t = nc.dram_tensor("wqT", [D, D], BF16, kind="ExternalInput")     # W_qkv.T
    wo_t = nc.dram_tensor("woT", [D, D], BF16, kind="ExternalInput")     # W_out.T
    temp_t = nc.dram_tensor("temp", [H, 1], F32, kind="ExternalInput")
    bias_t = nc.dram_tensor("bout", [1, D], BF16, kind="ExternalInput")
    selB_t = nc.dram_tensor("selB", [NT, P, NCH * P], BF16, kind="ExternalInput")
    selPB_t = nc.dram_tensor("selPB", [NT, P, NCH * P], BF16, kind="ExternalInput")
    blk16_t = nc.dram_tensor("blk16", [P, NCH], BF16, kind="ExternalInput")
    bsel8_t = nc.dram_tensor("bsel8", [NCH, P], F32R, kind="ExternalInput")
    selH_t = nc.dram_tensor("selH", [P, H], F32, kind="ExternalInput")
    sel16_t = nc.dram_tensor("sel16", [NT, H, P], F32, kind="ExternalInput")
    y_t = nc.dram_tensor("y", [N, D], F32, kind="ExternalOutput")

    with tile.TileContext(nc) as tc:
        with (
            tc.tile_pool(name="consts", bufs=1) as consts,
            tc.tile_pool(name="wsb", bufs=1) as wsb,
            tc.tile_pool(name="small", bufs=1) as small,
        ):
            # ---------- persistent big tensors ----------
            w_tiles = [wsb.tile([P, N], BF16, tag=f"w{t}", name=f"w{t}") for t in range(NT)]
            s_strip = [small.tile([P, NCH], F32, tag=f"ss{t}", name=f"ss{t}") for t in range(NT)]
            d_strip = [small.tile([P, NCH], F32, tag=f"ds{t}", name=f"ds{t}") for t in range(NT)]

            # right-side long-lived pools (independent release stack)
            wop = tc.alloc_tile_pool(name="wop", bufs=1, side="right")
            wo_sb = wop.tile([P, NT, D], BF16, tag="wo")
            soft = tc.alloc_tile_pool(name="soft", bufs=1, side="right")

            # selPB selectors live until MM2; w^2 tiles freed after dots pass
            selPBp = tc.alloc_tile_pool(name="selPBp", bufs=1)
            selPB = [
                selPBp.tile([P, NCH * P], BF16, tag=f"selPB{t}", name=f"selPB{t}")
                for t in range(NT)
            ]
            w2p = tc.alloc_tile_pool(name="w2p", bufs=1)
            w2_tiles = [w2p.tile([P, N], BF16, tag=f"w2_{t}", name=f"w2_{t}") for t in range(NT)]

            # ---------- weights for MM1 (freed after phase A) ----------
            wqp = tc.alloc_tile_pool(name="wqp", bufs=1)
            wq_sb = wqp.tile([P, NT, D], BF16, tag="wq")
            for k in range(NT):
                nc.sync.dma_start(out=wq_sb[:, k, :], in_=wq_t[k * P : (k + 1) * P, :])

            # ---------- small constants ----------
            temp_sb = consts.tile([H, 1], F32)
            nc.sync.dma_start(out=temp_sb, in_=temp_t[:, :])
            bias_bf = consts.tile([1, D], BF16)
            nc.sync.dma_start(out=bias_bf, in_=bias_t[:, :])
            ones1 = consts.tile([1, P], BF16)
            nc.vector.memset(ones1, 1.0)
            blk16 = consts.tile([P, NCH], BF16)
            nc.sync.dma_start(out=blk16, in_=blk16_t[:, :])
            bsel8 = consts.tile([NCH, P], F32R)
            nc.sync.dma_start(out=bsel8, in_=bsel8_t[:, :])
            selH = consts.tile([P, H], F32)
            nc.sync.dma_start(out=selH, in_=selH_t[:, :])
            sel16 = []
            for t in range(NT):
                s16 = consts.tile([H, P], F32, tag=f"s16_{t}", name=f"s16_{t}")
                nc.sync.dma_start(out=s16, in_=sel16_t[t, :, :])
                sel16.append(s16)

            # ---------- phase A: w.T = WqkvT.T @ x.T, w2, s ----------
            with (
                tc.tile_pool(name="xk", bufs=4, side="right") as xk,
                tc.tile_pool(name="psA", bufs=1, space="PSUM") as psA,
            ):
                for j in range(NCH):
                    js = slice(j * CH, (j + 1) * CH)
                    w_ps = [
                        psA.tile([P, CH], F32, tag=f"mm1_{t}", name=f"wps{t}")
                        for t in range(NT)
                    ]
                    for k in range(NT):
                        xT_k = xk.tile([P, CH], BF16, tag="xk")
                        nc.sync.dma_start(
                            out=xT_k, in_=xT_t[k * P : (k + 1) * P, js]
                        )
                        for t in range(NT):
                            nc.tensor.matmul(
                                w_ps[t],
                                wq_sb[:, k, t * P : (t + 1) * P],
                                xT_k,
                                start=(k == 0),
                                stop=(k == NT - 1),
                            )
                    for t in range(NT):
                        nc.scalar.copy(out=w_tiles[t][:, js], in_=w_ps[t])
                        nc.vector.scalar_tensor_tensor(
                            out=w2_tiles[t][:, js],
                            in0=w_tiles[t][:, js],
                            scalar=1.0,
                            in1=w_tiles[t][:, js],
                            op0=MUL,
                            op1=MUL,
                            accum_out=s_strip[t][:, j : j + 1],
                        )

            wqp.release()

            # selector consts + W_out.T arrive during the phase A tail
            selBp = tc.alloc_tile_pool(name="selBp", bufs=1)
            selB = []
            for t in range(NT):
                sb = selBp.tile([P, NCH * P], BF16, tag=f"selB{t}", name=f"selB{t}")
                nc.sync.dma_start(out=sb, in_=selB_t[t, :, :])
                selB.append(sb)
            for t in range(NT):
                nc.sync.dma_start(out=selPB[t], in_=selPB_t[t, :, :])
            for k in range(NT):
                nc.sync.dma_start(out=wo_sb[:, k, :], in_=wo_t[k * P : (k + 1) * P, :])

            # ---------- stats 1: inv_t = temp[h] / max(s, eps); lbig ----------
            with tc.tile_pool(name="psS1", bufs=2, space="PSUM") as psS1:
                for t in range(NT):
                    s_all = small.tile([P, 1], F32, tag=f"sall{t}")
                    nc.vector.reduce_sum(s_all, s_strip[t], axis=mybir.AxisListType.X)
                    nc.vector.tensor_scalar_max(out=s_all, in0=s_all, scalar1=1e-24)
                    rcp = small.tile([P, 1], F32, tag=f"rcp{t}")
                    nc.vector.reciprocal(rcp, s_all)
                    tb_ps = psS1.tile([P, 1], F32, tag="tb")
                    nc.tensor.matmul(tb_ps, sel16[t], temp_sb, start=True, stop=True)
                    inv_t = small.tile([P, 1], F32, tag=f"invt{t}")
                    nc.vector.tensor_mul(inv_t, rcp, tb_ps)
                    # lbig built in place: selB[t] *= inv_t (per-partition)
                    nc.vector.tensor_scalar_mul(
                        out=selB[t], in0=selB[t], scalar1=inv_t
                    )

            # ---------- phase L: banded logits L2[16j+h, m] ----------
            psL = tc.alloc_tile_pool(name="psL", bufs=1, space="PSUM")
            L2_ps = psL.tile([P, CH], F32, tag="L2")
            for j in range(NCH):
                js = slice(j * CH, (j + 1) * CH)
                for t in range(NT):
                    nc.tensor.matmul(
                        L2_ps,
                        selB[t][:, j * P : (j + 1) * P],
                        w2_tiles[t][:, js],
                        start=(j == 0 and t == 0),
                        stop=(j == NCH - 1 and t == NT - 1),
                    )
            selBp.release()

            # ---------- softmax over h (within 16-partition bands) ----------
            with tc.tile_pool(name="psSM", bufs=2, space="PSUM") as psSM:
                e2 = soft.tile([P, CH], BF16, tag="e2")
                nc.scalar.activation(out=e2, in_=L2_ps, func=EXP)
                s8_ps = psSM.tile([NCH, CH], F32, tag="s8")
                nc.tensor.matmul(s8_ps, blk16, e2, start=True, stop=True)
                lns = soft.tile([NCH, CH], F32R, tag="lns")
                nc.scalar.activation(out=lns, in_=s8_ps, func=LN)
                lnb_ps = psSM.tile([P, CH], F32, tag="lnb")
                nc.tensor.matmul(lnb_ps, bsel8, lns, start=True, stop=True)
                lnb_sb = soft.tile([P, CH], F32, tag="lnbsb")
                nc.scalar.copy(out=lnb_sb, in_=lnb_ps)
                d2 = soft.tile([P, CH], F32, tag="d2")
                nc.vector.tensor_sub(d2, L2_ps, lnb_sb)
                pi2 = soft.tile([P, CH], BF16, tag="pi2")
                nc.scalar.activation(out=pi2, in_=d2, func=EXP)
            psL.release()

            # sumpi -> ispi [16,1]; per-tile per-partition ispi
            isp_pp = []
            with tc.tile_pool(name="psSP", bufs=2, space="PSUM") as psSP:
                colsum = small.tile([P, 1], F32, tag="colsum")
                nc.vector.reduce_sum(colsum, pi2, axis=mybir.AxisListType.X)
                sp_ps = psSP.tile([H, 1], F32, tag="sp")
                nc.tensor.matmul(sp_ps, selH, colsum, start=True, stop=True)
                sumpi = small.tile([H, 1], F32, tag="sumpi")
                nc.vector.tensor_scalar_add(out=sumpi, in0=sp_ps, scalar1=1e-8)
                ispi = small.tile([H, 1], F32, tag="ispi")
                nc.vector.reciprocal(ispi, sumpi)
                for t in range(NT):
                    ip_ps = psSP.tile([P, 1], F32, tag="ipp")
                    nc.tensor.matmul(ip_ps, sel16[t], ispi, start=True, stop=True)
                    ipp = small.tile([P, 1], F32, tag=f"ipp{t}")
                    nc.vector.tensor_copy(ipp, ip_ps)
                    isp_pp.append(ipp)

            # ---------- dots pass: d_strip accumulation (pib read from PSUM) ----------
            with (
                tc.tile_pool(name="scrD", bufs=3) as scrD,
                tc.tile_pool(name="psP", bufs=3, space="PSUM") as psP,
            ):
                for j in range(NCH):
                    js = slice(j * CH, (j + 1) * CH)
                    for t in range(NT):
                        pib_ps = psP.tile([P, CH], F32, tag="pib")
                        nc.tensor.matmul(
                            pib_ps,
                            selPB[t][:, j * P : (j + 1) * P],
                            pi2,
                            start=True,
                            stop=True,
                        )
                        junk = scrD.tile([P, CH], BF16, tag="junkD")
                        nc.vector.scalar_tensor_tensor(
                            out=junk,
                            in0=w2_tiles[t][:, js],
                            scalar=isp_pp[t],
                            in1=pib_ps,
                            op0=MUL,
                            op1=MUL,
                            accum_out=d_strip[t][:, j : j + 1],
                        )
            w2p.release()

            # ---------- stats 2: attn_t = -1 / (1 + dots) per tile ----------
            attn_pp = []
            for t in range(NT):
                dots = small.tile([P, 1], F32, tag=f"dots{t}")
                nc.vector.reduce_sum(dots, d_strip[t], axis=mybir.AxisListType.X)
                nc.vector.tensor_scalar_add(out=dots, in0=dots, scalar1=1.0)
                attn = small.tile([P, 1], F32, tag=f"attn{t}")
                nc.vector.reciprocal(attn, dots)
                nc.vector.tensor_scalar_mul(out=attn, in0=attn, scalar1=-1.0)
                attn_pp.append(attn)

            # ---------- fused u + MM2: y = u.T @ WoutT + b ----------
            with (
                tc.tile_pool(name="upool", bufs=2) as upool,
                tc.tile_pool(name="och", bufs=2) as och,
                tc.tile_pool(name="psP2", bufs=3, space="PSUM") as psP2,
                tc.tile_pool(name="psM", bufs=4, space="PSUM") as psM,
            ):
                for j in range(NCH):
                    js = slice(j * CH, (j + 1) * CH)
                    uch = upool.tile([P, NT, CH], BF16, tag="uch")
                    for t in range(NT):
                        pib_ps = psP2.tile([P, CH], F32, tag="pib2")
                        nc.tensor.matmul(
                            pib_ps,
                            selPB[t][:, j * P : (j + 1) * P],
                            pi2,
                            start=True,
                            stop=True,
                        )
                        nc.vector.scalar_tensor_tensor(
                            out=uch[:, t, :],
                            in0=w_tiles[t][:, js],
                            scalar=attn_pp[t],
                            in1=pib_ps,
                            op0=MUL,
                            op1=MUL,
                        )
                    for half in range(2):
                        outf = och.tile([P, 2, D], F32, tag="outf")
                        for mh in range(2):
                            m = half * 2 + mh
                            msl = slice(m * P, (m + 1) * P)
                            for oh in range(2):
                                osl = slice(oh * CH, (oh + 1) * CH)
                                f_ps = psM.tile([P, CH], F32, tag="mm2")
                                nc.tensor.matmul(
                                    f_ps,
                                    ones1,
                                    bias_bf[:, osl],
                                    start=True,
                                    stop=False,
                                )
                                for t in range(NT):
                                    nc.tensor.matmul(
                                        f_ps,
                                        uch[:, t, msl],
                                        wo_sb[:, t, osl],
                                        start=False,
                                        stop=(t == NT - 1),
                                    )
                                nc.scalar.copy(out=outf[:, mh, osl], in_=f_ps)
                        nc.sync.dma_start(
                            out=y_t[
                                j * CH + half * (CH // 2) : j * CH + (half + 1) * (CH // 2),
                                :,
                            ].rearrange("(m p) i -> p m i", p=P),
                            in_=outf,
                        )
            selPBp.release()
            soft.release()
            wop.release()

    if not nc.is_finalized():
        nc.finalize()
    return nc


_NC_CACHE = None
_LAST_IN_MAPS = None
_RUNNER = None


def _make_runner(nc, n_cores):
    """Like bass2jax.run_bass_via_pjrt but with the jitted callable cached,
    so repeat calls don't re-trace/re-compile the XLA wrapper."""
    import jax
    from jax.experimental.shard_map import shard_map
    from jax.sharding import Mesh, PartitionSpec
    from concourse import mybir as _mybir
    from concourse.bass2jax import (
        _bass_exec_p,
        install_neuronx_cc_hook,
        partition_id_tensor,
    )

    install_neuronx_cc_hook()

    partition_name = nc.partition_id_tensor.name if nc.partition_id_tensor else None
    in_names, out_names, out_avals, zero_outs = [], [], [], []
    for alloc in nc.m.functions[0].allocations:
        if not isinstance(alloc, _mybir.MemoryLocationSet):
            continue
        name = alloc.memorylocations[0].name
        if alloc.kind == "ExternalInput":
            if name != partition_name:
                in_names.append(name)
        elif alloc.kind == "ExternalOutput":
            shape = tuple(alloc.tensor_shape)
            dtype = _mybir.dt.np(alloc.dtype)
            out_names.append(name)
            out_avals.append(jax.core.ShapedArray(shape, dtype))
            zero_outs.append(np.zeros(shape, dtype))
    n_params = len(in_names)
    n_outs = len(out_names)
    all_in_names = in_names + out_names + (
        [partition_name] if partition_name else []
    )
    donate = tuple(range(n_params, n_params + n_outs))

    def _body(*args):
        operands = list(args)
        if partition_name is not None:
            operands.append(partition_id_tensor())
        outs = _bass_exec_p.bind(
            *operands,
            out_avals=tuple(out_avals),
            in_names=tuple(all_in_names),
            out_names=tuple(out_names),
            lowering_input_output_aliases=(),
            sim_require_finite=True,
            sim_require_nnan=True,
            nc=nc,
        )
        return tuple(outs)

    devices = jax.devices()[:n_cores]
    mesh = Mesh(np.asarray(devices), ("core",))
    in_specs = (PartitionSpec("core"),) * (n_params + n_outs)
    out_specs = (PartitionSpec("core"),) * n_outs
    sharded = jax.jit(
        shard_map(
            _body, mesh=mesh, in_specs=in_specs, out_specs=out_specs, check_rep=False
        ),
        donate_argnums=donate,
        keep_unused=True,
    )

    def run(in_maps):
        concat_in = [
            np.concatenate([np.asarray(m[name]) for m in in_maps], axis=0)
            for name in in_names
        ]
        concat_zeros = [
            np.zeros((n_cores * z.shape[0], *z.shape[1:]), z.dtype)
            for z in zero_outs
        ]
        out_arrs = sharded(*concat_in, *concat_zeros)
        return {
            name: np.asarray(out_arrs[i]).reshape(n_cores, *out_avals[i].shape)
            for i, name in enumerate(out_names)
        }

    run.sharded = sharded
    run.meta = (in_names, out_names, out_avals, n_params, n_outs)
    return run


def _selector_consts():
    import ml_dtypes

    bf = ml_dtypes.bfloat16
    selB = np.zeros((NT, P, NCH * P), dtype=bf)
    selPB = np.zeros((NT, P, NCH * P), dtype=bf)
    for t in range(NT):
        for p in range(P):
            h = 2 * t + p // HD
            for j in range(NCH):
                selB[t, p, j * P + 16 * j + h] = 1.0
        for j in range(NCH):
            for c2 in range(P):
                h = 2 * t + c2 // HD
                selPB[t, 16 * j + h, j * P + c2] = 1.0
    blk16 = np.zeros((P, NCH), dtype=bf)
    for p in range(P):
        blk16[p, p // 16] = 1.0
    bsel8 = np.zeros((NCH, P), dtype=np.float32)
    for p in range(P):
        bsel8[p // 16, p] = 1.0
    selH = np.zeros((P, H), dtype=np.float32)
    for p in range(P):
        selH[p, p % 16] = 1.0
    sel16 = np.zeros((NT, H, P), dtype=np.float32)
    for t in range(NT):
        sel16[t, 2 * t, 0:HD] = 1.0
        sel16[t, 2 * t + 1, HD:P] = 1.0
    return selB, selPB, blk16, bsel8, selH, sel16


def kernel(x, W_qkv, temp, W_out, b_out):
    global _NC_CACHE, _RUNNER, _LAST_IN_MAPS
    import ml_dtypes

    bf = ml_dtypes.bfloat16
    if _NC_CACHE is None:
        _NC_CACHE = build()
        _RUNNER = _make_runner(_NC_CACHE, B)

    x = np.asarray(x, dtype=np.float32)
    wqT = np.asarray(W_qkv, dtype=np.float32).T.astype(bf)
    woT = np.asarray(W_out, dtype=np.float32).T.astype(bf)
    temp = np.ascontiguousarray(np.asarray(temp, dtype=np.float32).reshape(H, 1))
    bout = np.asarray(b_out, dtype=np.float32).reshape(1, D).astype(bf)

    selB, selPB, blk16, bsel8, selH, sel16 = _selector_consts()

    in_maps = [
        {
            "xT": np.ascontiguousarray(x[i].T).astype(bf),
            "wqT": wqT,
            "woT": woT,
            "temp": temp,
            "bout": bout,
            "selB": selB,
            "selPB": selPB,
            "blk16": blk16,
            "bsel8": bsel8,
            "selH": selH,
            "sel16": sel16,
        }
        for i in range(B)
    ]
    _LAST_IN_MAPS = in_maps
    out = _RUNNER(in_maps)
    return out["y"]


if __name__ == "__main__":
    rng = np.random.default_rng(0)
    x = rng.standard_normal((B, N, D), dtype=np.float32)
    W_qkv = (rng.standard_normal((D, D), dtype=np.float32) * 0.02).astype(np.float32)
    temp = np.ones((H, 1), dtype=np.float32)
    W_out = (rng.standard_normal((D, D), dtype=np.float32) * 0.02).astype(np.float32)
    b_out = np.zeros((D,), dtype=np.float32)
    y = kernel(x=x, W_qkv=W_qkv, temp=temp, W_out=W_out, b_out=b_out)
    print("kernel ran, y shape", y.shape, "mean abs", np.abs(y).mean())
